# revision 11
# baseline (speedup 1.0000x reference)
"""CSPN 3x3 propagation step on 8 trn2 NeuronCores (batch-parallel).

out[b, y, x] = sum_{t=(a,c)} guide[b, t, y+1, x+1] * src_t[...]
  src_t = hn shifted by (1-a, 1-c), center tap (a=c=1) uses h0.

Per core (B=1): row-shift accumulation is done on the TensorEngine with
exact 0/1 shift matrices (fp32 matmul is bit-exact for row selection),
column shifts are free-dim offsets on the DVE product ops.
"""

import numpy as np

import concourse.bacc as bacc
import concourse.mybir as mybir
from concourse import tile
from concourse.bass_utils import run_bass_kernel_spmd

F32 = mybir.dt.float32

B, H, W = 8, 352, 1216
HP, WP = H + 2, W + 2          # padded plane dims (354, 1218)
N_CORES = 8
# output row chunks: i0 = first padded out row, R = rows in chunk
CHUNKS = [(1, 126), (127, 126), (253, 100)]
# column strips (out padded col j0, width N)
STRIPS = [(1, 512), (513, 512), (1025, 192)]


def make_shift_mats():
    """S_d[k, m] = 1 iff k == m + d, for d in {0,1,2}; packed [128, 378]."""
    sm = np.zeros((128, 3 * 126), np.float32)
    for d in range(3):
        for m in range(126):
            sm[m + d, d * 126 + m] = 1.0
    return sm


def build(n_iters: int = 1):
    nc = bacc.Bacc()
    g_d = nc.dram_tensor("guide", [9, HP, WP], F32, kind="ExternalInput")
    hn_d = nc.dram_tensor("hn", [H, W], F32, kind="ExternalInput")
    h0_d = nc.dram_tensor("h0", [H, W], F32, kind="ExternalInput")
    sm_d = nc.dram_tensor("smat", [128, 3 * 126], F32, kind="ExternalInput")
    out_d = nc.dram_tensor("out", [H, W], F32, kind="ExternalOutput")

    with tile.TileContext(nc) as tc:
        with tc.tile_pool(name="const", bufs=1) as cpool, \
             tc.tile_pool(name="gpool", bufs=2) as gpool, \
             tc.tile_pool(name="spool", bufs=2) as spool, \
             tc.tile_pool(name="ppool", bufs=6) as ppool, \
             tc.tile_pool(name="opool", bufs=2) as opool, \
             tc.tile_pool(name="psum", bufs=2, space="PSUM") as pspool:

            smt = cpool.tile([128, 3 * 126], F32)
            nc.sync.dma_start(out=smt[:], in_=sm_d[:])

            # Split every load into two partition-halves, one per HWDGE
            # ring (SP + ACT sequencers), so both descriptor streams run
            # concurrently on every transfer. The GPSIMD SWDGE ring
            # measured ~2x slower — do not use it.
            def dual_dma(dst, src):
                n = dst.shape[0]
                m = n // 2
                if m == 0:
                    nc.sync.dma_start(out=dst, in_=src)
                    return
                nc.sync.dma_start(out=dst[0:m], in_=src[0:m])
                nc.scalar.dma_start(out=dst[m:n], in_=src[m:n])

            def chunk_body(i0, R):
                u0 = i0 - 1  # tile partition p <-> padded row u0 + p

                # ---- hn tile (padded plane window rows u0..u0+127)
                hnt = spool.tile([128, WP], F32, tag="hn")
                p_lo = max(0, 1 - u0)            # first partition with a real hn row
                p_hi = min(128, H + 1 - u0)      # one past last real row
                if p_hi < 128:
                    # bottom edge: zero the tail (covers pad row u=353) first
                    nc.vector.memset(hnt[96:128, :], 0.0)
                nc.vector.memset(hnt[:, 0:1], 0.0)
                nc.vector.memset(hnt[:, WP - 1:WP], 0.0)
                if p_lo > 0:
                    nc.vector.memset(hnt[0:1, :], 0.0)
                dual_dma(hnt[p_lo:p_hi, 1:WP - 1],
                         hn_d[u0 + p_lo - 1:u0 + p_hi - 1, :])

                # ---- h0 tile (same window; pads never selected)
                h0t = spool.tile([128, WP], F32, tag="h0")
                if p_lo > 0:
                    nc.vector.memset(h0t[0:1, :], 0.0)
                if p_hi < 128:
                    nc.vector.memset(h0t[96:128, :], 0.0)
                nc.vector.memset(h0t[:, 0:1], 0.0)
                nc.vector.memset(h0t[:, WP - 1:WP], 0.0)
                dual_dma(h0t[p_lo:p_hi, 1:WP - 1],
                         h0_d[u0 + p_lo - 1:u0 + p_hi - 1, :])

                # ---- guide tiles: tap t partition k holds g_t row u0+k-Di
                # Out-of-plane partitions are zeroed (never selected by the
                # shift matrices; zeroing keeps PSUM NaN-free and CoreSim
                # race-clean).
                gts = []
                for t in range(9):
                    a = t // 3
                    di = 1 - a
                    gt = gpool.tile([128, WP], F32, tag=f"g{t}")
                    lo = u0 - di
                    lo_c, hi_c = max(lo, 0), min(lo + 128, HP)
                    d0 = lo_c - lo
                    if d0 > 0:
                        nc.vector.memset(gt[0:1, :], 0.0)
                    if d0 + (hi_c - lo_c) < 128:
                        nc.vector.memset(gt[96:128, :], 0.0)
                    dual_dma(gt[d0:d0 + hi_c - lo_c, :], g_d[t, lo_c:hi_c, :])
                    gts.append(gt)

                # ---- products + shift-matmul accumulation
                psts = [pspool.tile([126, 512], F32, tag=f"ps{s}", name=f"ps{s}")
                        for s in range(len(STRIPS))]
                for t in range(9):
                    a, c = t // 3, t % 3
                    di = 1 - a
                    src = h0t if t == 4 else hnt
                    pt = ppool.tile([128, W], F32, tag="prod")
                    # P_t[k, w] = g_t[k, 1+w] * src[k, (2-c)+w]
                    cs = 1 if t == 4 else (2 - c)
                    nc.vector.tensor_tensor(pt[:, 0:W], gts[t][:, 1:1 + W],
                                            src[:, cs:cs + W],
                                            mybir.AluOpType.mult)
                    for s, (j0, N) in enumerate(STRIPS):
                        nc.tensor.matmul(psts[s][:, 0:N],
                                         smt[:, (di + 1) * 126:(di + 1) * 126 + 126],
                                         pt[:, j0 - 1:j0 - 1 + N],
                                         start=(t == 0), stop=(t == 8))

                # ---- PSUM -> SBUF -> HBM
                ot = opool.tile([126, W], F32, tag="out")
                for s, (j0, N) in enumerate(STRIPS):
                    nc.scalar.copy(out=ot[0:R, j0 - 1:j0 - 1 + N],
                                   in_=psts[s][0:R, 0:N])
                dual_dma(out_d[i0 - 1:i0 - 1 + R, :], ot[0:R, 0:W])

            def body(_iv=None):
                for i0, R in CHUNKS:
                    chunk_body(i0, R)

            if n_iters == 1:
                body()
            else:
                with tc.For_i(0, n_iters, 1):
                    body()

    nc.finalize()
    return nc


_nc_cache = {}


def _get_nc(n_iters=1):
    if n_iters not in _nc_cache:
        _nc_cache[n_iters] = build(n_iters)
    return _nc_cache[n_iters]


def kernel(guide_weight: np.ndarray, hn: np.ndarray, h0: np.ndarray) -> np.ndarray:
    """Full inputs: guide_weight [8,9,354,1218], hn/h0 [8,1,352,1216] f32.
    Returns [8,1,352,1216] f32."""
    nc = _get_nc(1)
    sm = make_shift_mats()
    in_maps = [
        {
            "guide": np.ascontiguousarray(guide_weight[b], dtype=np.float32),
            "hn": np.ascontiguousarray(hn[b, 0], dtype=np.float32),
            "h0": np.ascontiguousarray(h0[b, 0], dtype=np.float32),
            "smat": sm,
        }
        for b in range(B)
    ]
    res = run_bass_kernel_spmd(nc, in_maps, list(range(N_CORES)))
    out = np.stack([res.results[b]["out"] for b in range(B)], axis=0)
    return out[:, None].astype(np.float32)
